# revision 1
# baseline (speedup 1.0000x reference)
"""Trainium2 Bass kernel for nn_MI_35115652612725 (mutual-information loss).

Math (see reference): per h-slice,
  xs = softmax(x_seen[.,h]/T, -1)  (h, N, C1),  xu = softmax(x_unseen/T, -1)^T
  p_joint = xu @ xs / N;  p_seen/p_unseen are its column/row marginals.
  out = mean_h[ -sum p_joint*(log p_joint - log p_seen) + sum p_unseen*log p_unseen ]

Sharding: 8 cores = (h=4) x (v=2).  Core 2h+v processes the contiguous slabs
x_seen[v,h] (2048x1024) and x_unseen[v,h] (2048x2048); the N=v*B contraction
axis splits exactly along v.  Per-row softmax normalizers are folded into the
smaller matmul operand (xsw = exp_s/(ss*su*N)), so the bf16 matmul
pjT[k,c] = sum_n xsw[n,k]*eu[n,c] directly yields p_joint^T partials.  Two
pipelined ReduceScatters over core pairs {2h,2h+1} (one per half of the k
range) both sum the two v-partials and split the k axis for the entropy
phase, overlapping collective + entropy work with the remaining matmuls.
p_seen = rowsum of the final pjT rows (core-local); p_unseen partials ride
along in the RS buffers.  Each core emits two partial entropy scalars; the
host sums 16 scalars.

A fixed -90 shift replaces the per-row max in softmax: any per-row constant
cancels exactly in all outputs here, and with |20*x| < 115 neither exp nor
the f32 sums can overflow (underflow only kills terms < e^-60 of the row
max, far below f32 softmax resolution anyway).
"""

import numpy as np

import concourse.bass as bass
import concourse.bacc as bacc
import concourse.mybir as mybir
from concourse import tile
from concourse.tile import add_dep_helper
from concourse.bass_utils import run_bass_kernel_spmd

F32 = mybir.dt.float32
BF16 = mybir.dt.bfloat16
AF = mybir.ActivationFunctionType
ALU = mybir.AluOpType
AX = mybir.AxisListType

V, H, B, C1, C2 = 2, 4, 2048, 1024, 2048
N = V * B
P = 128
NT = B // P            # 16 row tiles of the contraction axis
KB = C1 // P           # 8 stationary blocks (k on psum partitions)
CH = C2 // 512         # 4 moving chunks (c on psum free axis)
SCALE = 20.0           # 1/TEMP
SHIFT = -90.0
EPS = 1e-7

NG = 4                 # pipelined collective groups (split along k)
KBG = KB // NG         # kb blocks per group (2)
SUBPJ = (KBG // 2) * P * C2   # per-rank pjT piece: 128*2048 elems
PUNW = C2 // NG // 2          # per-rank p_unseen piece: 256
SUBCH = SUBPJ + PUNW

_NC = None
LAST_RESULTS = None


def _build_nc():
    nc = bacc.Bacc(None, num_devices=8)
    # Register the exp shift as a preamble const AP (memset + barrier before
    # any tile instruction) so the Exp activations don't pick up an extra
    # sync-wait on a bias-producing instruction — the ACT instruction
    # encoding only has room for one wait here.
    shift_t = nc.alloc_sbuf_tensor(f"const-float32-{SHIFT}", [128, 1], F32)
    nc.gpsimd.memset(shift_t.ap(), SHIFT)
    nc.const_aps.aps[(F32, SHIFT)] = shift_t.ap()
    nc.all_engine_barrier()

    xs_d = nc.dram_tensor("xs", [B, C1], F32, kind="ExternalInput")
    xu_d = nc.dram_tensor("xu", [B, C2], F32, kind="ExternalInput")
    out_d = nc.dram_tensor("parts", [1, 2], F32, kind="ExternalOutput")

    with tile.TileContext(nc) as tc:
        with (
            tc.tile_pool(name="dram", bufs=1, space="DRAM") as dram,
            tc.tile_pool(name="xu_raw", bufs=2) as pool_xu,
            tc.tile_pool(name="xs_raw", bufs=2) as pool_xs,
            tc.tile_pool(name="es", bufs=2) as pool_es,
            tc.tile_pool(name="eu", bufs=NT) as pool_eu,
            tc.tile_pool(name="xsw", bufs=NT) as pool_xsw,
            tc.tile_pool(name="stat", bufs=8 * NT) as stat,
            tc.tile_pool(name="psum", bufs=6, space="PSUM") as psum,
            tc.tile_pool(name="psum_pu", bufs=1, space="PSUM") as psum_pu,
            tc.tile_pool(name="psum_fin", bufs=1, space="PSUM") as psum_fin,
            tc.tile_pool(name="pjt", bufs=2) as pool_pjt,
            tc.tile_pool(name="pcl", bufs=2) as pool_pcl,
            tc.tile_pool(name="lp", bufs=1) as pool_lp,
            tc.tile_pool(name="evict", bufs=4) as pool_ev,
            tc.tile_pool(name="evict_pu", bufs=2) as pool_evpu,
            tc.tile_pool(name="pu3", bufs=2) as pool_pu3,
            tc.tile_pool(name="acc", bufs=1) as acc,
        ):
            rs_in = [
                dram.tile([2 * SUBCH], BF16, name=f"rs_in{g}") for g in range(NG)
            ]
            rs_out = [dram.tile([SUBCH], BF16, name=f"rs_out{g}") for g in range(NG)]

            xu_t = xu_d[:].rearrange("(t p) c -> t p c", p=P)
            xs_t = xs_d[:].rearrange("(t p) c -> t p c", p=P)

            # ---------------- phase 1: exp + fold normalizers ----------------
            eu_tiles, xsw_tiles, ssw_tiles = [], [], []
            for t in range(NT):
                xu_raw = pool_xu.tile([P, C2], F32)
                nc.sync.dma_start(xu_raw[:], xu_t[t])
                eu = pool_eu.tile([P, C2], BF16, tag="eu", name=f"eu{t}")
                su = stat.tile([P, 1], F32, tag="stat", name=f"su{t}")
                nc.scalar.activation(
                    eu[:], xu_raw[:], AF.Exp, bias=SHIFT, scale=SCALE, accum_out=su[:]
                )

                xs_raw = pool_xs.tile([P, C1], F32)
                nc.sync.dma_start(xs_raw[:], xs_t[t])
                es = pool_es.tile([P, C1], BF16)
                ss = stat.tile([P, 1], F32, tag="stat", name=f"ss{t}")
                nc.scalar.activation(
                    es[:], xs_raw[:], AF.Exp, bias=SHIFT, scale=SCALE, accum_out=ss[:]
                )

                den = stat.tile([P, 1], F32, tag="stat", name=f"den{t}")
                nc.vector.tensor_tensor(den[:], su[:], ss[:], op=ALU.mult)
                den2 = stat.tile([P, 1], F32, tag="stat", name=f"den2_{t}")
                nc.vector.tensor_scalar_mul(den2[:], den[:], float(N))
                w = stat.tile([P, 1], F32, tag="stat", name=f"w{t}")
                nc.vector.reciprocal(w[:], den2[:])

                xsw = pool_xsw.tile([P, C1], BF16, tag="xsw", name=f"xsw{t}")
                nc.vector.tensor_scalar_mul(xsw[:], es[:], w[:])
                # row-sum of xsw is analytically ss*w = 1/(su*N)
                ssw_f = stat.tile([P, 1], F32, tag="stat", name=f"sswf{t}")
                nc.vector.tensor_tensor(ssw_f[:], ss[:], w[:], op=ALU.mult)
                ssw_b = stat.tile([P, 1], BF16, tag="stat", name=f"sswb{t}")
                nc.vector.tensor_copy(ssw_b[:], ssw_f[:])

                eu_tiles.append(eu)
                xsw_tiles.append(xsw)
                ssw_tiles.append(ssw_b)

            # ---------------- phase 2 + overlapped collectives ----------------
            # group g covers kb in [g*KBG, (g+1)*KBG) and pun cols
            # [g*1024, (g+1)*1024); sub-chunk r of group g holds pjT rows
            # [g*512 + r*256, +256) and pun [g*1024 + r*512, +512).
            last_ev = {}

            def emit_pj_group(g):
                for kb in range(g * KBG, (g + 1) * KBG):
                    ps_tiles = [
                        psum.tile([P, 512], F32, tag="pjps", name=f"pjps{kb}_{ch}")
                        for ch in range(CH)
                    ]
                    for t in range(NT):
                        lhsT = xsw_tiles[t][:, kb * P : (kb + 1) * P]
                        for ch in range(CH):
                            nc.tensor.matmul(
                                ps_tiles[ch][:],
                                lhsT,
                                eu_tiles[t][:, ch * 512 : (ch + 1) * 512],
                                start=(t == 0),
                                stop=(t == NT - 1),
                            )
                    r = kb - g * KBG
                    pj_view = rs_in[g][r * SUBCH : r * SUBCH + SUBPJ].rearrange(
                        "(k c) -> k c", c=C2
                    )
                    for ch in range(CH):
                        ev = pool_ev.tile([P, 512], BF16, tag="ev")
                        last_ev["copy"] = nc.vector.tensor_copy(ev[:], ps_tiles[ch][:])
                        last_ev["dma"] = nc.sync.dma_start(
                            pj_view[:, ch * 512 : (ch + 1) * 512], ev[:]
                        )
                # p_unseen partials for this group's 512-wide chunk
                pu_ps = psum_pu.tile([1, 512], F32, tag="pups", name=f"pups{g}")
                for t in range(NT):
                    nc.tensor.matmul(
                        pu_ps[:],
                        ssw_tiles[t][:],
                        eu_tiles[t][:, g * 512 : (g + 1) * 512],
                        start=(t == 0),
                        stop=(t == NT - 1),
                    )
                ev = pool_evpu.tile([1, 512], BF16, tag="evpu")
                last_ev["copy"] = nc.vector.tensor_copy(ev[:], pu_ps[:])
                for r in range(2):
                    pun_view = rs_in[g][
                        r * SUBCH + SUBPJ : r * SUBCH + SUBPJ + PUNW
                    ].rearrange("(a c) -> a c", a=1)
                    last_ev["dma"] = nc.sync.dma_start(
                        pun_view[:], ev[:, r * PUNW : (r + 1) * PUNW]
                    )

                nc.gpsimd.collective_compute(
                    "ReduceScatter",
                    ALU.add,
                    replica_groups=[[0, 1], [2, 3], [4, 5], [6, 7]],
                    ins=[rs_in[g].opt()],
                    outs=[rs_out[g].opt()],
                )

            # -------------------- phase 3: entropies --------------------
            ones = acc.tile([P, 1], F32)
            nc.vector.memset(ones[:], 1.0)
            s1c = acc.tile([P, NG], F32)
            s2g = acc.tile([1, NG], F32)

            def emit_entropy_group(g):
                # order-only deps: keep the in-order DVE / SP queues free of
                # collective-dependent entropy work until every PSUM eviction
                # (which feeds the PE) has issued, else PE stalls behind the
                # collectives (head-of-line blocking).
                def after_ev(inst):
                    add_dep_helper(inst.ins, last_ev["copy"].ins, sync=False,
                                   reason="entropy after evictions")
                    return inst

                def after_ev_dma(inst):
                    add_dep_helper(inst.ins, last_ev["dma"].ins, sync=False,
                                   reason="entropy dma after eviction dmas")
                    return inst

                pj_t = pool_pjt.tile([P, C2], BF16, tag="pjt", name=f"pjt{g}")
                after_ev_dma(nc.sync.dma_start(
                    pj_t[:], rs_out[g][0:SUBPJ].rearrange("(p c) -> p c", c=C2)
                ))
                psn = stat.tile([P, 1], F32, tag="stat", name=f"psn{g}")
                after_ev(nc.vector.reduce_sum(psn[:], pj_t[:], axis=AX.X))
                psc = stat.tile([P, 1], F32, tag="stat", name=f"psc{g}")
                nc.vector.tensor_scalar_max(psc[:], psn[:], EPS)
                lps = stat.tile([P, 1], F32, tag="stat", name=f"lps{g}")
                nc.scalar.activation(lps[:], psc[:], AF.Ln)

                rs_cl = stat.tile([P, 1], F32, tag="stat", name=f"rscl{g}")
                pcl = pool_pcl.tile([P, C2], F32)
                after_ev(nc.vector.tensor_scalar(
                    pcl[:], pj_t[:], EPS, None, op0=ALU.max, op1=ALU.add,
                    accum_out=rs_cl[:],
                ))
                lp = pool_lp.tile([P, C2], F32)
                nc.scalar.activation(lp[:], pcl[:], AF.Ln)
                # NOTE: tensor_tensor_reduce wedges the exec unit on this
                # runtime (NRT_EXEC_UNIT_UNRECOVERABLE) — mult in place into
                # pcl, then reduce.
                a1 = stat.tile([P, 1], F32, tag="stat", name=f"a1_{g}")
                nc.vector.tensor_tensor(pcl[:], pcl[:], lp[:], op=ALU.mult)
                nc.vector.reduce_sum(a1[:], pcl[:], axis=AX.X)
                b1 = stat.tile([P, 1], F32, tag="stat", name=f"b1_{g}")
                nc.vector.tensor_tensor(b1[:], lps[:], rs_cl[:], op=ALU.mult)
                nc.vector.tensor_tensor(
                    s1c[:, g : g + 1], a1[:], b1[:], op=ALU.subtract
                )

                # p_unseen entropy for this group's final slice
                puf = pool_pu3.tile([1, PUNW], BF16, tag="puf", name=f"puf{g}")
                after_ev_dma(nc.sync.dma_start(
                    puf[:],
                    rs_out[g][SUBPJ : SUBPJ + PUNW].rearrange("(a c) -> a c", a=1),
                ))
                puc = pool_pu3.tile([1, PUNW], F32, tag="puc", name=f"puc{g}")
                after_ev(nc.vector.tensor_scalar_max(puc[:], puf[:], EPS))
                lpu = pool_pu3.tile([1, PUNW], F32, tag="lpu", name=f"lpu{g}")
                nc.scalar.activation(lpu[:], puc[:], AF.Ln)
                pup = pool_pu3.tile([1, PUNW], F32, tag="pup", name=f"pup{g}")
                nc.vector.tensor_tensor(pup[:], puc[:], lpu[:], op=ALU.mult)
                nc.vector.reduce_sum(s2g[:, g : g + 1], pup[:], axis=AX.X)

            # all matmul groups first: the DVE stream must finish every PSUM
            # eviction before any collective-dependent entropy op, or the
            # in-order DVE queue stalls the PE behind the collectives.
            for g in range(NG):
                emit_pj_group(g)
            for g in range(NG):
                emit_entropy_group(g)

            # cross-partition total of s1, then emit [s1, s2]
            s1r = acc.tile([P, 1], F32)
            nc.vector.reduce_sum(s1r[:], s1c[:], axis=AX.X)
            ps_fin = psum_fin.tile([1, 1], F32, tag="fin")
            nc.tensor.matmul(ps_fin[:], s1r[:], ones[:])
            s2 = acc.tile([1, 1], F32)
            nc.vector.reduce_sum(s2[:], s2g[:], axis=AX.X)
            fin = acc.tile([1, 2], F32)
            nc.scalar.copy(fin[:, 0:1], ps_fin[:])
            nc.vector.tensor_copy(fin[:, 1:2], s2[:])
            nc.sync.dma_start(out_d[:], fin[:])

    nc.finalize()
    return nc


def _get_nc():
    global _NC
    if _NC is None:
        _NC = _build_nc()
    return _NC


def make_in_maps(x_seen, x_unseen):
    in_maps = []
    for h in range(H):
        for v in range(V):
            in_maps.append(
                {
                    "xs": np.ascontiguousarray(x_seen[v, h]),
                    "xu": np.ascontiguousarray(x_unseen[v, h]),
                }
            )
    return in_maps


def kernel(x_seen: np.ndarray, x_unseen: np.ndarray) -> np.ndarray:
    import os

    global LAST_RESULTS
    nc = _get_nc()
    in_maps = make_in_maps(x_seen, x_unseen)
    trace = os.environ.get("KERNEL_TRACE", "0") == "1"
    kw = {}
    td = os.environ.get("KERNEL_TRACE_DIR")
    if td:
        kw["tmpdir"] = td
    res = run_bass_kernel_spmd(nc, in_maps, list(range(H * V)), trace=trace, **kw)
    LAST_RESULTS = res
    s1 = sum(float(r["parts"][0, 0]) for r in res.results)
    s2 = sum(float(r["parts"][0, 1]) for r in res.results)
    return np.array((s2 - s1) / H, dtype=np.float32)

